# revision 8
# baseline (speedup 1.0000x reference)
"""Trainium2 Bass kernel for masked-product attention (sparse_attention).

Reference computes, per (batch, head):
    scores      = (Q @ K^T) / 8 * (MQ @ MK^T) / 8          # [S, S]
    p_attn      = softmax(scores, axis=-1)                 # [S, S]
    p_val       = p_attn @ V                               # [S, D]
with B=2, H=16, S=2048, D=64 (fp32).

Sharding: B*H = 32 head-slots are split across 8 NeuronCores (4 per core);
each core computes its heads fully independently (no collectives).

Per-core layout (all compute in "transposed" [k, q] orientation):
  - Inputs are cast fp32->bf16 by SWDGE DMA into a DRAM staging buffer
    [2048, 128] = [seq, Qd | MQd], then xbar-DMA-transposed into SBUF
    as [128, 2048] = [Qd | MQd (stacked on partitions), seq].
  - Scores^T chunk [k_tile=128, q]: content matmul uses partitions 0:64
    (d-contraction), mask matmul partitions 64:128 -> the two matmuls
    occupy distinct PE row-groups and run concurrently.
  - DVE multiplies the two PSUM score tiles (fp32) -> t = s*m.
  - ACT computes E = exp(t/64) -> bf16 (scale folded into activation).
  - PV: U'[d|rowsum, q] accumulates V'^T @ E^T in PSUM over the 16
    k-chunks, where V' = [V | ones] so row 64 of U' is the softmax
    denominator for free.
  - reciprocal -> broadcast across partitions with a K=1 ones matmul ->
    p_attn^T = E^T * recip (bf16), p_val^T = U' * recip (fp32).
Outputs are written transposed ([h][k][q] and [h][d][q]); the host
returns a cheap numpy transposed view.
"""

import sys
from contextlib import ExitStack

import numpy as np

sys.path.insert(0, "/opt/trn_rl_repo")

import ml_dtypes  # noqa: E402

from concourse import bacc, mybir  # noqa: E402
from concourse import bass  # noqa: E402
from concourse import tile  # noqa: E402
from concourse.bass_utils import run_bass_kernel_spmd  # noqa: E402

B, H, S, D = 2, 16, 2048, 64
N_CORES = 8
HPC = (B * H) // N_CORES          # heads per core = 4
KC = S // 128                     # k chunks = 16
F32 = mybir.dt.float32
BF16 = mybir.dt.bfloat16
FP16 = mybir.dt.float16
EXPF = mybir.ActivationFunctionType.Exp
MUL = mybir.AluOpType.mult


def _build_graph():
    nc = bacc.Bacc("TRN2", target_bir_lowering=False, debug=False)

    ins = {
        name: nc.dram_tensor(name, [HPC, S, D], F32, kind="ExternalInput")
        for name in ("query", "key", "value", "m_query", "m_key")
    }
    stage_qmq = nc.dram_tensor("stage_qmq", [HPC, S, 128], BF16)
    stage_kmk = nc.dram_tensor("stage_kmk", [HPC, S, 128], BF16)
    p_attn_t = nc.dram_tensor("p_attn_t", [HPC, S, S], BF16, kind="ExternalOutput")
    p_val_t = nc.dram_tensor("p_val_t", [HPC, D, S], F32, kind="ExternalOutput")

    with tile.TileContext(nc) as tc, ExitStack() as ctx:
        sb = ctx.enter_context(tc.tile_pool(name="sb", bufs=2))
        tp = ctx.enter_context(tc.tile_pool(name="t", bufs=3))
        mp = ctx.enter_context(tc.tile_pool(name="m", bufs=3))
        ep = ctx.enter_context(tc.tile_pool(name="E", bufs=12))
        pp = ctx.enter_context(tc.tile_pool(name="P", bufs=3))
        rp = ctx.enter_context(tc.tile_pool(name="r", bufs=2))
        cp = ctx.enter_context(tc.tile_pool(name="const", bufs=1))
        pss_pool = ctx.enter_context(
            tc.tile_pool(name="pss", bufs=2, space=bass.MemorySpace.PSUM)
        )
        psm_pool = ctx.enter_context(
            tc.tile_pool(name="psm", bufs=1, space=bass.MemorySpace.PSUM)
        )
        psu = ctx.enter_context(
            tc.tile_pool(name="psu", bufs=1, space=bass.MemorySpace.PSUM)
        )

        ones = cp.tile([1, 128], F32)
        nc.vector.memset(ones[:], 1.0)

        for h in range(HPC):
            # ---- stage inputs: cast to bf16, interleave content|mask ----
            nc.gpsimd.dma_start(out=stage_qmq[h, :, 0:64], in_=ins["query"][h])
            nc.gpsimd.dma_start(out=stage_qmq[h, :, 64:128], in_=ins["m_query"][h])
            nc.gpsimd.dma_start(out=stage_kmk[h, :, 0:64], in_=ins["key"][h])
            nc.gpsimd.dma_start(out=stage_kmk[h, :, 64:128], in_=ins["m_key"][h])

            qmqt = sb.tile([128, S], BF16, tag="qmqt")   # [Qd|MQd, q]
            nc.sync.dma_start_transpose(out=qmqt[:], in_=stage_qmq[h])
            kmkt = sb.tile([128, S], BF16, tag="kmkt")   # [Kd|MKd, k]
            nc.sync.dma_start_transpose(out=kmkt[:], in_=stage_kmk[h])

            # V' = [V | ones] per k-chunk: [128, KC, 65] bf16
            vt = sb.tile([128, KC, 65], BF16, tag="vt")
            nc.gpsimd.dma_start(
                out=vt[:, :, 0:64],
                in_=ins["value"][h].rearrange("(c p) d -> p c d", p=128),
            )
            nc.vector.memset(vt[:, :, 64:65], 1.0)

            for half in range(2):
                q0 = half * 1024
                pu = psu.tile([65, 1024], F32)           # U' accum (2 banks)
                e_tiles = []   # one [128, 2048] tile per chunk PAIR (c, c+1)
                for c in range(KC):
                    k0 = c * 128
                    pss = pss_pool.tile([128, 1024], F32, tag="pss")
                    psm = psm_pool.tile([128, 1024], F32, tag="psm")
                    for j in range(2):
                        qq = q0 + j * 512
                        # content scores^T: rows 0:64 of PE array
                        nc.tensor.matmul(
                            pss[:, j * 512:(j + 1) * 512],
                            kmkt[0:64, k0:k0 + 128],
                            qmqt[0:64, qq:qq + 512], start=True, stop=True,
                        )
                        # mask scores^T: rows 64:128, runs concurrently
                        nc.tensor.matmul(
                            psm[:, j * 512:(j + 1) * 512],
                            kmkt[64:128, k0:k0 + 128],
                            qmqt[64:128, qq:qq + 512], start=True, stop=True,
                        )
                    # mask scores leave PSUM (TT cannot read two PSUM operands)
                    msb = mp.tile([128, 1024], FP16, tag="m")
                    nc.scalar.copy(msb[:], psm[:])
                    if c % 2 == 0:
                        tt = tp.tile([128, 2048], F32, tag="t")
                    nc.vector.tensor_tensor(
                        tt[:, (c % 2) * 1024:(c % 2 + 1) * 1024], pss[:], msb[:], MUL
                    )
                    if c % 2 == 1:
                        et = ep.tile([128, 2048], BF16, tag="E")
                        nc.scalar.activation(et[:], tt[:], EXPF, scale=1.0 / 64.0)
                        e_tiles.append(et)
                        for cc in (c - 1, c):
                            esl = et[:, (cc % 2) * 1024:(cc % 2 + 1) * 1024]
                            for j in range(2):
                                nc.tensor.matmul(
                                    pu[0:65, j * 512:(j + 1) * 512],
                                    vt[:, cc, :], esl[:, j * 512:(j + 1) * 512],
                                    start=(cc == 0), stop=(cc == KC - 1),
                                )

                # ---- normalization for this (head, q-half) ----
                r_sb = rp.tile([1, 1024], F32, tag="r")
                nc.vector.reciprocal(r_sb[:], pu[64:65, :])
                rb_ps = psm_pool.tile([128, 1024], F32, tag="psm")
                for j in range(2):
                    nc.tensor.matmul(
                        rb_ps[:, j * 512:(j + 1) * 512],
                        ones[0:1, :], r_sb[0:1, j * 512:(j + 1) * 512],
                        start=True, stop=True,
                    )
                rb16 = rp.tile([128, 1024], BF16, tag="rb16")
                nc.scalar.copy(rb16[:], rb_ps[:])
                rb32 = rp.tile([128, 1024], F32, tag="rb32")
                nc.scalar.copy(rb32[:], rb_ps[:])
                pv_sb = rp.tile([64, 1024], F32, tag="pv")
                nc.vector.tensor_tensor(
                    pv_sb[:], pu[0:64, :], rb32[0:64, :], MUL
                )
                nc.sync.dma_start(
                    out=p_val_t[h, :, q0:q0 + 1024], in_=pv_sb[:]
                )
                for ci, et in enumerate(e_tiles):
                    pc = pp.tile([128, 2048], BF16, tag="P")
                    for cc in (2 * ci, 2 * ci + 1):
                        psl = pc[:, (cc % 2) * 1024:(cc % 2 + 1) * 1024]
                        nc.vector.tensor_tensor(
                            psl, et[:, (cc % 2) * 1024:(cc % 2 + 1) * 1024],
                            rb16[:], MUL,
                        )
                        nc.sync.dma_start(
                            out=p_attn_t[h, cc * 128:(cc + 1) * 128, q0:q0 + 1024],
                            in_=psl,
                        )

    nc.compile()
    return nc


_NC = None
_LAST_IN_MAPS = None


def _get_nc():
    global _NC
    if _NC is None:
        _NC = _build_graph()
    return _NC


def profile_exec_ns():
    """Re-run the last kernel() inputs with NTFF tracing; return exec_time_ns."""
    if _LAST_IN_MAPS is None:
        return None
    nc = _get_nc()
    try:
        res = run_bass_kernel_spmd(
            nc, _LAST_IN_MAPS, core_ids=list(range(N_CORES)), trace=True
        )
        return res.exec_time_ns
    except Exception as e:  # hook missing under some axon builds
        print(f"profile failed: {e}")
        return None


def time_exec_ns(iters=3):
    """Wall-clock the SPMD execute (includes PJRT dispatch overhead)."""
    import time
    if _LAST_IN_MAPS is None:
        return None
    nc = _get_nc()
    run_bass_kernel_spmd(nc, _LAST_IN_MAPS, core_ids=list(range(N_CORES)))
    best = None
    for _ in range(iters):
        t0 = time.perf_counter()
        run_bass_kernel_spmd(nc, _LAST_IN_MAPS, core_ids=list(range(N_CORES)))
        dt = (time.perf_counter() - t0) * 1e9
        best = dt if best is None else min(best, dt)
    return int(best)


def kernel(query, key, value, m_query, m_key):
    nc = _get_nc()
    full = {
        "query": query, "key": key, "value": value,
        "m_query": m_query, "m_key": m_key,
    }
    flat = {
        k: np.ascontiguousarray(
            np.asarray(v, dtype=np.float32).reshape(B * H, S, D)
        )
        for k, v in full.items()
    }
    in_maps = [
        {k: np.ascontiguousarray(v[i * HPC:(i + 1) * HPC]) for k, v in flat.items()}
        for i in range(N_CORES)
    ]
    global _LAST_IN_MAPS
    _LAST_IN_MAPS = in_maps
    res = run_bass_kernel_spmd(nc, in_maps, core_ids=list(range(N_CORES)))

    p_val = np.empty((B * H, S, D), dtype=np.float32)
    p_attn_t = np.empty((B * H, S, S), dtype=np.float32)
    for i, r in enumerate(res.results):
        pvt = np.asarray(r["p_val_t"])                       # [HPC, D, S] f32
        pat = np.asarray(r["p_attn_t"])                      # [HPC, S(k), S(q)] bf16
        for j in range(HPC):
            p_val[i * HPC + j] = pvt[j].T
        p_attn_t[i * HPC:(i + 1) * HPC] = pat.astype(np.float32)
    p_val = p_val.reshape(B, H, S, D)
    # stored [h][k][q]; reference wants [h][q][k] -> transposed view
    p_attn = p_attn_t.reshape(B, H, S, S).transpose(0, 1, 3, 2)
    return (p_val, p_attn)


# revision 13
# speedup vs baseline: 17055.2374x; 17055.2374x over previous
"""Trainium2 Bass kernel for masked-product attention (sparse_attention).

Reference computes, per (batch, head):
    scores      = (Q @ K^T) / 8 * (MQ @ MK^T) / 8          # [S, S]
    p_attn      = softmax(scores, axis=-1)                 # [S, S]
    p_val       = p_attn @ V                               # [S, D]
with B=2, H=16, S=2048, D=64 (fp32).

Sharding: B*H = 32 head-slots are split across 8 NeuronCores (4 per core);
each core computes its heads fully independently (no collectives).

Per-core layout (all compute in "transposed" [k, q] orientation):
  - Inputs are cast fp32->bf16 by SWDGE DMA into a DRAM staging buffer
    [2048, 128] = [seq, Qd | MQd], then xbar-DMA-transposed into SBUF
    as [128, 2048] = [Qd | MQd (stacked on partitions), seq].
  - Scores^T chunk [k_tile=128, q]: content matmul uses partitions 0:64
    (d-contraction), mask matmul partitions 64:128 -> the two matmuls
    occupy distinct PE row-groups and run concurrently.
  - DVE multiplies the two PSUM score tiles (fp32) -> t = s*m.
  - ACT computes E = exp(t/64) -> bf16 (scale folded into activation).
  - PV: U'[d|rowsum, q] accumulates V'^T @ E^T in PSUM over the 16
    k-chunks, where V' = [V | ones] so row 64 of U' is the softmax
    denominator for free.
  - reciprocal -> broadcast across partitions with a K=1 ones matmul ->
    p_attn^T = E^T * recip (bf16), p_val^T = U' * recip (fp32).
Outputs are written transposed ([h][k][q] and [h][d][q]); the host
returns a cheap numpy transposed view.
"""

import sys
from contextlib import ExitStack

import numpy as np

sys.path.insert(0, "/opt/trn_rl_repo")

import ml_dtypes  # noqa: E402

from concourse import bacc, mybir  # noqa: E402
from concourse import bass  # noqa: E402
from concourse import tile  # noqa: E402
from concourse.bass_utils import run_bass_kernel_spmd  # noqa: E402

B, H, S, D = 2, 16, 2048, 64
N_CORES = 8
HPC = (B * H) // N_CORES          # heads per core = 4
KC = S // 128                     # k chunks = 16
F32 = mybir.dt.float32
BF16 = mybir.dt.bfloat16
FP16 = mybir.dt.float16
EXPF = mybir.ActivationFunctionType.Exp
MUL = mybir.AluOpType.mult


def _build_graph():
    nc = bacc.Bacc("TRN2", target_bir_lowering=False, debug=False)

    ins = {
        name: nc.dram_tensor(name, [HPC, S, D], F32, kind="ExternalInput")
        for name in ("query", "key", "value", "m_query", "m_key")
    }
    stage_qmq = nc.dram_tensor("stage_qmq", [HPC, S, 128], BF16)
    stage_kmk = nc.dram_tensor("stage_kmk", [HPC, S, 128], BF16)
    r_sum_d = nc.dram_tensor("r_sum_d", [HPC * 2, 1024], F32)
    r_rec_d = nc.dram_tensor("r_rec_d", [HPC * 2, 1024], F32)
    p_attn_t = nc.dram_tensor("p_attn_t", [HPC, S, S], BF16, kind="ExternalOutput")
    p_val_t = nc.dram_tensor("p_val_t", [HPC, D, S], F32, kind="ExternalOutput")

    with tile.TileContext(nc) as tc, ExitStack() as ctx:
        sb = ctx.enter_context(tc.tile_pool(name="sb", bufs=2))
        tp = ctx.enter_context(tc.tile_pool(name="t", bufs=3))
        mp = ctx.enter_context(tc.tile_pool(name="m", bufs=3))
        ep = ctx.enter_context(tc.tile_pool(name="E", bufs=12))
        pp = ctx.enter_context(tc.tile_pool(name="P", bufs=3))
        rp = ctx.enter_context(tc.tile_pool(name="r", bufs=2))
        pss_pool = ctx.enter_context(
            tc.tile_pool(name="pss", bufs=2, space=bass.MemorySpace.PSUM)
        )
        psm_pool = ctx.enter_context(
            tc.tile_pool(name="psm", bufs=1, space=bass.MemorySpace.PSUM)
        )
        psu = ctx.enter_context(
            tc.tile_pool(name="psu", bufs=1, space=bass.MemorySpace.PSUM)
        )

        for h in range(HPC):
            # ---- stage inputs: cast to bf16, interleave content|mask ----
            nc.gpsimd.dma_start(out=stage_qmq[h, :, 0:64], in_=ins["query"][h])
            nc.gpsimd.dma_start(out=stage_qmq[h, :, 64:128], in_=ins["m_query"][h])
            nc.gpsimd.dma_start(out=stage_kmk[h, :, 0:64], in_=ins["key"][h])
            nc.gpsimd.dma_start(out=stage_kmk[h, :, 64:128], in_=ins["m_key"][h])

            qmqt = sb.tile([128, S], BF16, tag="qmqt")   # [Qd|MQd, q]
            nc.sync.dma_start_transpose(out=qmqt[:], in_=stage_qmq[h])
            kmkt = sb.tile([128, S], BF16, tag="kmkt")   # [Kd|MKd, k]
            nc.sync.dma_start_transpose(out=kmkt[:], in_=stage_kmk[h])

            # V' = [V | ones] per k-chunk: [128, KC, 65] bf16
            vt = sb.tile([128, KC, 65], BF16, tag="vt")
            nc.gpsimd.dma_start(
                out=vt[:, :, 0:64],
                in_=ins["value"][h].rearrange("(c p) d -> p c d", p=128),
            )
            nc.vector.memset(vt[:, :, 64:65], 1.0)

            for half in range(2):
                q0 = half * 1024
                pu = psu.tile([65, 1024], F32)           # U' accum (2 banks)
                e_tiles = []   # one [128, 2048] tile per chunk PAIR (c, c+1)
                for c in range(KC):
                    k0 = c * 128
                    pss = pss_pool.tile([128, 1024], F32, tag="pss")
                    psm = psm_pool.tile([128, 1024], F32, tag="psm")
                    for j in range(2):
                        qq = q0 + j * 512
                        # content scores^T: rows 0:64 of PE array
                        nc.tensor.matmul(
                            pss[:, j * 512:(j + 1) * 512],
                            kmkt[0:64, k0:k0 + 128],
                            qmqt[0:64, qq:qq + 512], start=True, stop=True,
                        )
                        # mask scores^T: rows 64:128, runs concurrently
                        nc.tensor.matmul(
                            psm[:, j * 512:(j + 1) * 512],
                            kmkt[64:128, k0:k0 + 128],
                            qmqt[64:128, qq:qq + 512], start=True, stop=True,
                        )
                    # mask scores leave PSUM (TT cannot read two PSUM operands)
                    msb = mp.tile([128, 1024], FP16, tag="m")
                    nc.scalar.copy(msb[:], psm[:])
                    if c % 2 == 0:
                        tt = tp.tile([128, 2048], F32, tag="t")
                    nc.vector.tensor_tensor(
                        tt[:, (c % 2) * 1024:(c % 2 + 1) * 1024], pss[:], msb[:], MUL
                    )
                    if c % 2 == 1:
                        et = ep.tile([128, 2048], BF16, tag="E")
                        nc.scalar.activation(et[:], tt[:], EXPF, scale=1.0 / 64.0)
                        e_tiles.append(et)
                        for cc in (c - 1, c):
                            esl = et[:, (cc % 2) * 1024:(cc % 2 + 1) * 1024]
                            for j in range(2):
                                nc.tensor.matmul(
                                    pu[0:65, j * 512:(j + 1) * 512],
                                    vt[:, cc, :], esl[:, j * 512:(j + 1) * 512],
                                    start=(cc == 0), stop=(cc == KC - 1),
                                )

                # ---- normalization for this (head, q-half) ----
                # rowsum row (psum, partition 64) -> sbuf -> DRAM -> respread
                # across 128 lanes -> reciprocal -> DRAM -> broadcast.
                vh = h * 2 + half
                r_sb = rp.tile([1, 1024], F32, tag="r")
                nc.scalar.copy(r_sb[:], pu[64:65, :])
                nc.sync.dma_start(out=r_sum_d[vh], in_=r_sb[:])
                rsp = rp.tile([128, 8], F32, tag="rsp")
                nc.sync.dma_start(
                    out=rsp[:], in_=r_sum_d[vh].rearrange("(p c) -> p c", p=128)
                )
                rrec = rp.tile([128, 8], F32, tag="rrec")
                nc.vector.reciprocal(rrec[:], rsp[:])
                nc.sync.dma_start(
                    out=r_rec_d[vh].rearrange("(p c) -> p c", p=128), in_=rrec[:]
                )
                rd_ap = r_rec_d[vh]
                bcast_src = bass.AP(rd_ap.tensor, rd_ap.offset, [[0, 128], [1, 1024]])
                rb32 = rp.tile([128, 1024], F32, tag="rb32")
                nc.sync.dma_start(out=rb32[:], in_=bcast_src)
                rb16 = rp.tile([128, 1024], BF16, tag="rb16")
                nc.gpsimd.dma_start(out=rb16[:], in_=bcast_src)
                pv_sb = rp.tile([64, 1024], F32, tag="pv")
                nc.vector.tensor_tensor(
                    pv_sb[:], pu[0:64, :], rb32[0:64, :], MUL
                )
                nc.sync.dma_start(
                    out=p_val_t[h, :, q0:q0 + 1024], in_=pv_sb[:]
                )
                for ci, et in enumerate(e_tiles):
                    pc = pp.tile([128, 2048], BF16, tag="P")
                    for cc in (2 * ci, 2 * ci + 1):
                        psl = pc[:, (cc % 2) * 1024:(cc % 2 + 1) * 1024]
                        nc.vector.tensor_tensor(
                            psl, et[:, (cc % 2) * 1024:(cc % 2 + 1) * 1024],
                            rb16[:], MUL,
                        )
                        nc.sync.dma_start(
                            out=p_attn_t[h, cc * 128:(cc + 1) * 128, q0:q0 + 1024],
                            in_=psl,
                        )

    nc.compile()
    return nc


_NC = None
_LAST_IN_MAPS = None


def _get_nc():
    global _NC
    if _NC is None:
        _NC = _build_graph()
    return _NC


def profile_exec_ns(trace_cores=(0,), keep_dir=False):
    """Re-run the last kernel() inputs with NTFF tracing; return exec_time_ns.

    Drives the axon NTFF capture directly via ctypes (the antenv.axon_hooks
    shim is absent in this image) and processes the NTFF locally with gauge.
    """
    if _LAST_IN_MAPS is None:
        return None
    import glob
    import tempfile

    import gauge.profiler
    from trn_agent_boot.trn_boot import _ntff_profile_via_ctypes
    from concourse import bass2jax
    from concourse._compat import FishPath
    from concourse.bass_utils import _process_ntff_profile

    nc = _get_nc()
    hook = _ntff_profile_via_ctypes("/opt/axon/libaxon_pjrt.so")
    if hook is None:
        print("profile: libaxon too old, no NTFF symbols")
        return None
    neff_dir = tempfile.mkdtemp(prefix="attn_prof_")
    with hook(neff_dir, list(trace_cores)):
        bass2jax.run_bass_via_pjrt(nc, _LAST_IN_MAPS, n_cores=N_CORES)
    ntffs = glob.glob(neff_dir + "/*_body*.ntff")
    if not ntffs:
        print(f"profile: no *_body*.ntff in {neff_dir}: {sorted(glob.glob(neff_dir + '/*'))}")
        return None
    profile = gauge.profiler.Profile(
        profile_path=FishPath(neff_dir),
        kernel_dev_mode=True,
        profile_on_exit=False,
        bass_kernel=nc.m,
        offline_processing=True,
        fname="*_body*",
    )
    res = _process_ntff_profile(
        profile, neff_dir, nc, list(range(N_CORES)),
        list(trace_cores), False, {}, trace_events=False,
    )
    if keep_dir:
        print(f"profile dir: {neff_dir}")
    return res.exec_time_ns


def time_exec_ns(iters=3):
    """Wall-clock the SPMD execute (includes PJRT dispatch overhead)."""
    import time
    if _LAST_IN_MAPS is None:
        return None
    nc = _get_nc()
    run_bass_kernel_spmd(nc, _LAST_IN_MAPS, core_ids=list(range(N_CORES)))
    best = None
    for _ in range(iters):
        t0 = time.perf_counter()
        run_bass_kernel_spmd(nc, _LAST_IN_MAPS, core_ids=list(range(N_CORES)))
        dt = (time.perf_counter() - t0) * 1e9
        best = dt if best is None else min(best, dt)
    return int(best)


def kernel(query, key, value, m_query, m_key):
    nc = _get_nc()
    full = {
        "query": query, "key": key, "value": value,
        "m_query": m_query, "m_key": m_key,
    }
    flat = {
        k: np.ascontiguousarray(
            np.asarray(v, dtype=np.float32).reshape(B * H, S, D)
        )
        for k, v in full.items()
    }
    in_maps = [
        {k: np.ascontiguousarray(v[i * HPC:(i + 1) * HPC]) for k, v in flat.items()}
        for i in range(N_CORES)
    ]
    global _LAST_IN_MAPS
    _LAST_IN_MAPS = in_maps
    res = run_bass_kernel_spmd(nc, in_maps, core_ids=list(range(N_CORES)))

    p_val = np.empty((B * H, S, D), dtype=np.float32)
    p_attn_t = np.empty((B * H, S, S), dtype=np.float32)
    for i, r in enumerate(res.results):
        pvt = np.asarray(r["p_val_t"])                       # [HPC, D, S] f32
        pat = np.asarray(r["p_attn_t"])                      # [HPC, S(k), S(q)] bf16
        for j in range(HPC):
            p_val[i * HPC + j] = pvt[j].T
        p_attn_t[i * HPC:(i + 1) * HPC] = pat.astype(np.float32)
    p_val = p_val.reshape(B, H, S, D)
    # stored [h][k][q]; reference wants [h][q][k] -> transposed view
    p_attn = p_attn_t.reshape(B, H, S, S).transpose(0, 1, 3, 2)
    return (p_val, p_attn)
